# revision 42
# baseline (speedup 1.0000x reference)
"""Trainium2 Bass kernel for nn_AEQMPatchSegmModel (retrieval_knn).

Strategy
--------
Data-parallel over batch: 16 images / 2 cores = 8 images per core,
processed as 4 serial image-pairs.  Two cores, not eight: the axon
protocol's per-call latency grows ~2-3 ms per participating device
(measured 1/2/4/8-core medians 56/48/54/69 ms with an identical NEFF),
while the whole batch is only ~2 ms of device compute — so fewer, busier
cores win.

The per-patch encoder (bilinear resize 8->16 + three stride-2 SAME convs +
dense) is an alternation of LINEAR maps and relus.  Each linear stage is
folded (on host, exactly) into a position-blocked matrix:

    A1 [2048,192]  = conv1 o resize   (patch pixels -> 8x8x32 pre-relu)
    A2 [1024,2048] = conv2            (8x8x32 -> 4x4x64 pre-relu)
    A3 [ 512,1024] = conv3            (4x4x64 -> 2x2x128 pre-relu)
    wd [ 512,  64] = dense

Conv locality makes these matrices block-sparse: an output row-block only
depends on 2-3 input row-blocks.  The device kernel runs a static schedule
of 128-wide accumulating matmuls over just the nonzero blocks, with
patches (n-tiles of 405 = 9 patch-rows x 45) on the matmul free dim.
Per-stage biases ride on the ACT engine (per-partition bias AP of the
relu/exp activations), so no ones-row / rank-1 bias matmuls are needed.

The RBF head is folded into one augmented matmul: V_aug = [v; v^2] (the
dense matmul emits v twice; one half is squared), then
  t[k,n] = sum_d 2*c_x[k,d]/s2 * v - 1/s2 * v^2  (+ bias: -|c_x|^2/s2 + log w_k)
  ow = exp(t) = comp_w * K^2;   sum_n max(ow,EPS) = sum_n relu(ow-EPS) + n*EPS
Per-core output: S [128 comps, 2 images].  Host: + n*EPS, normalize,
project through normalize(c_y)^2 -> (16,10).

Transport: the axon link to the NeuronCores has ~90 ms RTT and ~80 MB/s
bandwidth, so per-call traffic dominates wall time.  All weight-derived
constants are committed to the devices ONCE (cached jax Arrays keyed by a
hash of the weights); per call only the raw images ship, as a padded
x-parity-split layout IMG[g, c, x%2, 1+y, x//2] (2,3,2,100,48 per core).
The patch-major column-phase-expanded SBUF layout the A1 matmuls need is
built on-device by 24 strided DMAs from that DRAM image.
"""

import hashlib

import numpy as np

EPS = 1e-10
NTILE = 405          # 9 patch-rows x 45 patch-cols per n-tile
NT = 5               # n-tiles per image
NPI = 2025           # patches per image
NCORES = 2           # axon per-call latency scales with device count
NIMG = 16 // NCORES  # images per core
F32 = np.float32

_CACHE = {}


# ---------------------------------------------------------------- host math
def _resize_mat():
    R = np.zeros((16, 8), np.float64)
    for k in range(16):
        x = (k + 0.5) / 2.0 - 0.5
        x0 = int(np.floor(x))
        t = x - x0
        i0 = min(max(x0, 0), 7)
        i1 = min(max(x0 + 1, 0), 7)
        R[k, i0] += 1.0 - t
        R[k, i1] += t
    return R


def _conv_s2_same_jac(H, w):
    """Jacobian of jax stride-2 SAME 3x3 conv on (H,H,Cin) -> (H/2,H/2,Cout).

    XLA SAME for even H, stride 2, k=3: pad_total=1 -> pad_lo=0, pad_hi=1,
    so input row of output oy, tap ky is iy = 2*oy + ky (dropped if iy >= H).
    """
    kh, kw, Cin, Cout = w.shape
    Ho = H // 2
    J = np.zeros((Ho, Ho, Cout, H, H, Cin), np.float64)
    for ky in range(3):
        for kx in range(3):
            for oy in range(Ho):
                iy = 2 * oy + ky
                if iy >= H:
                    continue
                for ox in range(Ho):
                    ix = 2 * ox + kx
                    if ix >= H:
                        continue
                    J[oy, ox, :, iy, ix, :] += w[ky, kx].T
    return J.reshape(Ho * Ho * Cout, H * H * Cin)


def _a2_schedule():
    """[(mtile_index, kt1, my, p, jy, hh)] for conv2 block matmuls."""
    sched = []
    for my in range(4):
        for p in range(2):
            mt2 = my * 2 + p
            jys = [jy for jy in (2 * my, 2 * my + 1, 2 * my + 2) if jy <= 7]
            halves = (0, 1) if p == 0 else (1,)
            for jy in jys:
                for hh in halves:
                    sched.append((mt2, jy * 2 + hh, my, p, jy, hh))
    return sched


def _a3_schedule():
    """[(mtile_index, kt2, ny, nx, my, p)] for conv3 block matmuls."""
    sched = []
    for ny in range(2):
        for nx in range(2):
            mt3 = ny * 2 + nx
            mys = [my for my in (2 * ny, 2 * ny + 1, 2 * ny + 2) if my <= 3]
            pairs = (0, 1) if nx == 0 else (1,)
            for my in mys:
                for p in pairs:
                    sched.append((mt3, my * 2 + p, ny, nx, my, p))
    return sched


# a1c K-row permutation: p_new(s, jp=j%2, ci, jh=j//2) groups rows so each
# on-device gather DMA covers one (s, jp, ci) block of 4 jh-partitions,
# with jh stride-1 in the shipped image (jh and px share the xh axis).
def _a1_perm():
    perm = np.zeros(72, np.int64)
    for s in range(3):
        for j in range(8):
            for ci in range(3):
                perm[s * 24 + (j % 2) * 12 + ci * 4 + (j // 2)] = (
                    s * 24 + j * 3 + ci)
    return perm


def _build_consts(w1, b1, w2, b2, w3, b3, wd, bd, c_x, c_y, comp_w, sigma):
    R = _resize_mat()
    RZ = np.kron(np.kron(R, R), np.eye(3))                      # (768,192)
    A1 = _conv_s2_same_jac(16, np.asarray(w1, np.float64)) @ RZ  # (2048,192)
    A2 = _conv_s2_same_jac(8, np.asarray(w2, np.float64))        # (1024,2048)
    A3 = _conv_s2_same_jac(4, np.asarray(w3, np.float64))        # (512,1024)

    A1r = A1.reshape(8, 8, 32, 8, 8, 3)      # (jy,jx,co, i,j,ci)
    # merged A1: one K=72 matmul per (jy,h): rows (s,j,ci) with i = jy-1+s
    # baked against a 3x row-shift-replicated image
    a1c = np.zeros((72, 8, 2, 128), np.float64)
    chk = np.zeros_like(A1r)
    for jy in range(8):
        for h in range(2):
            for i in (jy - 1, jy, jy + 1):
                if not 0 <= i <= 7:
                    continue
                s = i - jy + 1
                blk = A1r[jy, 4 * h:4 * h + 4, :, i, :, :]       # (jx,co,j,ci)
                a1c[24 * s:24 * s + 24, jy, h, :] = (
                    blk.transpose(2, 3, 0, 1).reshape(24, 128))
                chk[jy, 4 * h:4 * h + 4, :, i, :, :] = blk
    assert np.abs(A1r - chk).max() < 1e-12, "A1 support mismatch"
    a1c = a1c[_a1_perm()]                    # device partition order

    s2 = _a2_schedule()
    A2r = A2.reshape(4, 4, 64, 8, 8, 32)     # (my,mx,co, jy,jx,ci)
    a2c = np.zeros((128, len(s2), 128), np.float64)
    chk = np.zeros_like(A2r)
    for n, (mt2, kt1, my, p, jy, hh) in enumerate(s2):
        blk = A2r[my, 2 * p:2 * p + 2, :, jy, 4 * hh:4 * hh + 4, :]  # (mx,co,jx,ci)
        a2c[:, n, :] = blk.transpose(2, 3, 0, 1).reshape(128, 128)
        chk[my, 2 * p:2 * p + 2, :, jy, 4 * hh:4 * hh + 4, :] = blk
    assert np.abs(A2r - chk).max() < 1e-12, "A2 support mismatch"

    s3 = _a3_schedule()
    A3r = A3.reshape(2, 2, 128, 4, 4, 64)    # (ny,nx,co, my,mx,ci)
    a3c = np.zeros((128, len(s3), 128), np.float64)
    chk = np.zeros_like(A3r)
    for n, (mt3, kt2, ny, nx, my, p) in enumerate(s3):
        blk = A3r[ny, nx, :, my, 2 * p:2 * p + 2, :]             # (co,mx,ci)
        a3c[:, n, :] = blk.transpose(1, 2, 0).reshape(128, 128)
        chk[ny, nx, :, my, 2 * p:2 * p + 2, :] = blk
    assert np.abs(A3r - chk).max() < 1e-12, "A3 support mismatch"

    wdm = np.asarray(wd, np.float64).reshape(4, 128, 64)         # (pos,c3,enc)
    wd2 = np.zeros((128, 4, 128), np.float64)
    wd2[:, :, 0:64] = wdm.transpose(1, 0, 2)
    wd2[:, :, 64:128] = wdm.transpose(1, 0, 2)

    # Fold the dense bias into the RBF head: with u = wd.T@h3 (pre-bias),
    # v = u + bd, so d2 = |u - (c_x - bd)|^2.  The device then never adds
    # bd; vaug = [u; u^2] and e := c_x - bd replaces c_x below.
    sig2 = float(np.asarray(sigma, np.float64) ** 2)
    e = np.asarray(c_x, np.float64) - np.asarray(bd, np.float64)[None, :]
    rbfw = np.zeros((128, 128), np.float64)
    rbfw[0:64, :] = (2.0 / sig2) * e.T
    rbfw[64:128, :] = -1.0 / sig2
    biasr = (-(e ** 2).sum(-1) / sig2
             + np.log(np.asarray(comp_w, np.float64)))           # (128,)

    # per-partition activation biases: conv1 (4x tiled b1), conv2 (2x b2),
    # conv3 (b3), rbf (biasr)
    bcols = np.zeros((128, 4), np.float64)
    bcols[:, 0] = np.tile(np.asarray(b1, np.float64), 4)
    bcols[:, 1] = np.tile(np.asarray(b2, np.float64), 2)
    bcols[:, 2] = np.asarray(b3, np.float64)
    bcols[:, 3] = biasr
    c = {
        "a1c": a1c, "a2c": a2c, "a3c": a3c, "wd2": wd2, "rbfw": rbfw,
        "bcols": bcols,
    }
    return {k: np.ascontiguousarray(v, F32) for k, v in c.items()}, s2, s3


def _weight_blob(consts):
    """[128, W1] per-core SBUF weight image (constant across calls)."""
    n2 = consts["a2c"].shape[1]
    n3 = consts["a3c"].shape[1]
    a1blk = np.zeros((128, 2048), F32)
    a1blk[0:72] = consts["a1c"].reshape(72, 2048)
    return np.ascontiguousarray(np.concatenate([
        consts["a2c"].reshape(128, n2 * 128),
        consts["a3c"].reshape(128, n3 * 128),
        consts["wd2"].reshape(128, 512),
        consts["rbfw"],
        consts["bcols"],
        a1blk,
    ], axis=1), F32)


def _prep_images(images):
    """(16,96,96,3) -> gather-friendly layout (8, 3, 2, 3, 2, 98, 48) f16.

    out[pair, s, jp, ci, g, y, xh] = padded image row s+y-1, col 2*xh+jp,
    channel ci of image 2*pair+g (row -1 and rows 96.. are zero).  The s
    axis replicates the three 98-row windows the conv taps need; xh is
    innermost so the on-device gather is 18 stride-1 DMAs per pair.

    f16 transport: ~5e-4 relative rounding on uniform[0,1) pixels, far
    inside the 2e-2 gate even for non-saturating weight regimes.
    """
    im = np.asarray(images, np.float16)
    pad = np.zeros((8, 2, 100, 48, 2, 3), np.float16)  # [pair,g,y,xh,jp,ci]
    pad[:, :, 1:97] = im.reshape(8, 2, 96, 48, 2, 3)
    out = np.empty((8, 3, 2, 3, 2, 98, 48), np.float16)
    for s in range(3):
        out[:, s] = pad[:, :, s:s + 98].transpose(0, 4, 5, 1, 2, 3)
    return out


# ---------------------------------------------------------------- device
def _build_nc(n2, n3, wb):
    import concourse.bass as bass
    import concourse.mybir as mybir
    import concourse.tile as tile
    import concourse.tile_sem_assignment as tsa
    from concourse.ap import AP
    tsa.NUM_HWDGE_SEMS = 1   # all HWDGE DMAs share one sem (kernel-tail
    #                          Drain has a tiny sync-wait budget in codegen).
    #                          One sem serializes DMA completions, so the
    #                          gather below is shaped into 6 big DMAs/pair.
    from concourse.vector_clock import ScopedClock

    def _split_drain_and_barrier(self, tick_clock, wait_clock):
        # codegen allows ~1 sync-wait per instruction; the stock tail drain
        # carries one wait per live semaphore.  Emit standalone SP waits
        # instead (drain first, then waits, then barrier — same net sync).
        bnc = self.nc
        drain_inst = bnc.sync.drain()
        wait_clock.add_sem_waits(
            drain_inst.ins, ScopedClock({None: tick_clock.global_clock})
        )
        si = drain_inst.ins.sync_info
        waits = list(si.on_wait) if si is not None and si.on_wait else []
        if len(waits) > 1:
            try:
                si.on_wait = waits[:0]
            except Exception:
                drain_inst.ins.sync_info = None
            num2sem = {s.num: s for s in self.sems.allocated().values()}
            for w in waits:
                bnc.sync.wait_ge(num2sem[int(w.id)], int(w.wait_value))
        bnc.all_engine_barrier()
        assert self.sems is not None
        popped = bnc._tile_sem_poison_stack.pop()
        assert popped is self._sem_poison
        bnc.clear_and_free_semaphores(list(self.sems.allocated().values()))
        bnc.all_engine_barrier()

    tile.TileContext._drain_and_barrier = _split_drain_and_barrier

    f32 = mybir.dt.float32
    AF = mybir.ActivationFunctionType
    nc = bass.Bass()
    _negeps = nc.alloc_sbuf_tensor("const-float32-negeps", [128, 1], f32)
    nc.gpsimd.memset(_negeps.ap(), -EPS)
    nc.const_aps.aps[(f32, -EPS)] = _negeps.ap()
    nc.all_engine_barrier()
    S_raw = nc.alloc_sbuf_tensor("Sout", [128, NIMG], f32)

    f16 = mybir.dt.float16
    W1 = (n2 + n3 + 4 + 1) * 128 + 4 + 2048    # wblob cols
    assert wb.shape == (128, W1) and wb.dtype == np.float32
    # weights baked into the NEFF as a Const tensor: loaded to HBM at model
    # load time, so per-call only the image crosses the axon link.
    # img layout [pair, s, jp, ci, g, y, xh] = padded image row s+y-1,
    # col 2*xh+jp, channel ci of image 2*pair+g (s-replicated, xh inner:
    # one DMA per (pair,s,jp,ci) moves 4 jh-partitions x 196 x 45 with a
    # stride-1 partition dim and merged (g,y)).
    wblob_d = nc.inline_tensor(wb, "wblob")
    img_d = nc.declare_dram_parameter(
        "img", [NIMG // 2, 3, 2, 3, 2, 98, 48], f16, isOutput=False)
    out_d = nc.declare_dram_parameter("out", [128, NIMG], f32, isOutput=True)

    s2 = _a2_schedule()
    s3 = _a3_schedule()

    with tile.TileContext(nc) as tc:
        with (
            tc.tile_pool(name="w", bufs=1) as wpool,
            tc.tile_pool(name="act", bufs=1) as apool,
            tc.tile_pool(name="sm", bufs=3) as spool,
            tc.tile_pool(name="ps", bufs=7, space="PSUM") as ppool,
        ):
            wblob = wpool.tile([128, W1], f32)
            nc.sync.dma_start(wblob[:], wblob_d[:])
            o = 0
            a2 = wblob[:, o:o + n2 * 128].rearrange("p (n k) -> p n k", n=n2)
            o += n2 * 128
            a3 = wblob[:, o:o + n3 * 128].rearrange("p (n k) -> p n k", n=n3)
            o += n3 * 128
            wdt = wblob[:, o:o + 512].rearrange("p (n k) -> p n k", n=4)
            o += 512
            rbf = wblob[:, o:o + 128]
            o += 128
            bias1 = wblob[:, o:o + 1]
            bias2 = wblob[:, o + 1:o + 2]
            bias3 = wblob[:, o + 2:o + 3]
            rbfb = wblob[:, o + 3:o + 4]
            o += 4
            a1 = wblob[0:72, o:o + 2048].rearrange(
                "p (jy h c) -> p jy h c", jy=8, h=2)

            # ACT pre-touch: makes the ACT clock observe the wblob DMA so
            # bias-AP reads below carry no extra sync-wait.
            dact = apool.tile([1, 1], f32, tag="dact")
            nc.scalar.activation(dact[:], wblob[0:1, 0:1], AF.Copy,
                                 bias=0.0, scale=1.0)
            # PE pre-touch: dummy matmul so the PE vector clock observes
            # the wblob load-DMA before the real matmuls (PE LDWEIGHTS
            # supports only ONE sync-wait slot in codegen).  Image-pair
            # tiles need no pre-touch: each pair's first matmul waits only
            # on the ACT upcast, and the serial pair structure keeps every
            # other cross-engine dep transitively observed.
            dps = ppool.tile([1, 1], f32, tag="dps", bufs=1)
            nc.tensor.matmul(dps[:], wblob[0:1, 0:1], wblob[0:1, 0:1],
                             start=True, stop=True)

            for pair in range(NIMG // 2):
                # On-device patch gather: IMG3h[(s,jp,jh,ci), (g,rr,par,px)]
                # = img[pair, s, jp, g, 2rr+par, jh+px, ci].
                # bufs = n_pairs: every pair gathers into a FRESH buffer, so
                # no gather DMA ever carries a cross-engine (WAR) wait on
                # top of its chain wait — codegen allows only one sync-wait
                # per DMACopy.
                IMG3h = apool.tile([72, 8820], f16, tag="img3h",
                                   bufs=NIMG // 2)
                for s in range(3):
                    for jp in range(2):
                        for ci in range(3):
                            p0 = s * 24 + jp * 12 + ci * 4
                            # src dims: jh(4)@1, (g,y)(196)@48, px(45)@1 —
                            # jh and px overlap on the xh axis, so the AP
                            # is hand-built
                            off = (((pair * 3 + s) * 2 + jp) * 3 + ci) * 9408
                            src = AP(img_d, off,
                                     [[1, 4], [48, 196], [1, 45]])
                            dst = IMG3h[p0:p0 + 4, :].rearrange(
                                "p (gy px) -> p gy px", px=45)
                            nc.sync.dma_start(dst, src)
                red = apool.tile([128, 2, 5], f32, tag="red", bufs=1)
                for g in range(2):
                    # f16 -> f32 upcast, one image at a time (half-size f32
                    # buffer keeps SBUF inside budget; the overwrite's WAR
                    # on the previous image's A1 reads is transitively
                    # observed through the relu chain, so no extra wait)
                    IMG3 = apool.tile([72, 4410], f32, tag="img3", bufs=1)
                    nc.scalar.activation(
                        IMG3[:], IMG3h[:, g * 4410:(g + 1) * 4410],
                        AF.Copy, bias=0.0, scale=1.0)
                    img = IMG3[:].rearrange(
                        "p (rr par px) -> p rr par px", rr=49, par=2)
                    _g_body(nc, mybir, ppool, apool, spool, img, red, g,
                            a1, a2, a3, wdt, rbf, bias1, bias2, bias3, rbfb,
                            s2, s3)
                junk = apool.tile([128, 5], f32, tag="junk")
                for g in range(2):
                    nc.scalar.activation(
                        junk[:], red[:, g, :], AF.Copy, bias=0.0, scale=1.0,
                        accum_out=S_raw.ap()[:, 2 * pair + g:2 * pair + g + 1])
    # Final DMA outside the TileContext: the tile drain+barrier already
    # synced everything, so this needs no Tile-tracked waits (codegen here
    # allows only one sync-wait per instruction).
    with nc.semaphore("out_sem") as out_sem:
        nc.sync.dma_start(out_d[:], S_raw.ap()).then_inc(out_sem, 16)
        nc.sync.wait_ge(out_sem, 16)
    return nc


def _g_body(nc, mybir, ppool, apool, spool, img, red, g,
            a1, a2, a3, wdt, rbf, bias1, bias2, bias3, rbfb, s2, s3):
    """Encoder + RBF head for one image (5 n-tiles of 405 patches)."""
    f32 = mybir.dt.float32
    AF = mybir.ActivationFunctionType
    if True:
            if True:
                for t in range(5):
                    # ---- A1: 8x8x32 pre-relu, M-tiles (jy, half) ----
                    # bufs=1: the relu writing h1(t+1) only runs after PE
                    # has executed A1-mm(t+1), which already follows every
                    # pair-t read of h1 — the WAR wait is always satisfied
                    h1 = apool.tile([128, 16, 405], f32, tag="h1", bufs=1)
                    for jy in range(8):
                        r0 = 18 * t + jy
                        for h in range(2):
                            ps = ppool.tile([128, 405], f32, tag="ps")
                            rhs = img[:, r0 // 2:r0 // 2 + 9, r0 % 2, :]
                            nc.tensor.matmul(
                                ps[:], a1[:, jy, h, :], rhs,
                                start=True, stop=True,
                            )
                            nc.scalar.activation(
                                h1[:, jy * 2 + h, :], ps[:], AF.Relu,
                                bias=bias1, scale=1.0,
                            )
                    # ---- A2: 4x4x64, M-tiles (my, mx-pair) ----
                    h2 = apool.tile([128, 8, 405], f32, tag="h2", bufs=2)
                    for mt in range(8):
                        idxs = [n for n, e in enumerate(s2) if e[0] == mt]
                        ps = ppool.tile([128, 405], f32, tag="ps")
                        for k, n in enumerate(idxs):
                            nc.tensor.matmul(
                                ps[:], a2[:, n, :], h1[:, s2[n][1], :],
                                start=(k == 0), stop=(k == len(idxs) - 1),
                            )
                        nc.scalar.activation(
                            h2[:, mt, :], ps[:], AF.Relu,
                            bias=bias2, scale=1.0,
                        )
                    # ---- A3: 2x2x128, M-tiles (ny,nx) ----
                    h3 = apool.tile([128, 4, 405], f32, tag="h3")
                    for mt in range(4):
                        idxs = [n for n, e in enumerate(s3) if e[0] == mt]
                        ps = ppool.tile([128, 405], f32, tag="ps")
                        for k, n in enumerate(idxs):
                            nc.tensor.matmul(
                                ps[:], a3[:, n, :], h2[:, s3[n][1], :],
                                start=(k == 0), stop=(k == len(idxs) - 1),
                            )
                        nc.scalar.activation(
                            h3[:, mt, :], ps[:], AF.Relu,
                            bias=bias3, scale=1.0,
                        )
                    # ---- dense -> [v; v] then v_aug = [v; v^2] ----
                    psv = ppool.tile([128, 405], f32, tag="ps")
                    for pos in range(4):
                        nc.tensor.matmul(
                            psv[:], wdt[:, pos, :], h3[:, pos, :],
                            start=(pos == 0), stop=(pos == 3),
                        )
                    vaug = spool.tile([128, 405], f32, tag="vaug")
                    nc.scalar.activation(
                        vaug[0:64, :], psv[0:64, :], AF.Copy, bias=0.0, scale=1.0
                    )
                    nc.scalar.activation(
                        vaug[64:128, :], psv[64:128, :], AF.Square,
                        bias=0.0, scale=1.0,
                    )
                    # ---- RBF + exp + eps-floor + reduce ----
                    psr = ppool.tile([128, 405], f32, tag="ps")
                    nc.tensor.matmul(psr[:], rbf[:], vaug[:], start=True,
                                     stop=True)
                    ow = spool.tile([128, 405], f32, tag="ow")
                    nc.scalar.activation(
                        ow[:], psr[:], AF.Exp, bias=rbfb, scale=1.0
                    )
                    # floor at EPS via relu(ow - EPS); ACT accum_out gives the
                    # per-partition sum over the 405 patches in one op
                    owr = spool.tile([128, 405], f32, tag="owr")
                    nc.scalar.activation(
                        owr[:], ow[:], AF.Relu, bias=-EPS, scale=1.0,
                        accum_out=red[:, g, t:t + 1],
                    )


# ---------------------------------------------------------------- runner
def _make_runner(nc):
    """jit(shard_map(bass_exec)) over 8 cores, mirroring
    bass2jax.run_bass_via_pjrt but reusable with device-committed args."""
    import jax
    import jax.core
    from jax.experimental.shard_map import shard_map
    from jax.sharding import Mesh, NamedSharding, PartitionSpec

    from concourse import bass2jax as b2j
    from concourse import mybir

    b2j.install_neuronx_cc_hook()
    partition_name = (nc.partition_id_tensor.name
                      if nc.partition_id_tensor else None)
    in_names, out_names, out_avals, zero_shapes = [], [], [], []
    for alloc in nc.m.functions[0].allocations:
        if not isinstance(alloc, mybir.MemoryLocationSet):
            continue
        name = alloc.memorylocations[0].name
        if alloc.kind == "ExternalInput":
            if name != partition_name:
                in_names.append(name)
        elif alloc.kind == "ExternalOutput":
            out_names.append(name)
            shape = tuple(alloc.tensor_shape)
            dtype = mybir.dt.np(alloc.dtype)
            out_avals.append(jax.core.ShapedArray(shape, dtype))
            zero_shapes.append((shape, dtype))
    n_params = len(in_names)
    all_in_names = in_names + out_names + (
        [partition_name] if partition_name else [])
    donate = tuple(range(n_params, n_params + len(out_names)))

    def _body(*args):
        operands = list(args)
        if partition_name is not None:
            operands.append(b2j.partition_id_tensor())
        outs = b2j._bass_exec_p.bind(
            *operands,
            out_avals=tuple(out_avals),
            in_names=tuple(all_in_names),
            out_names=tuple(out_names),
            lowering_input_output_aliases=(),
            sim_require_finite=True,
            sim_require_nnan=True,
            nc=nc,
        )
        return tuple(outs)

    devices = jax.devices()[:NCORES]
    assert len(devices) == NCORES, (
        f"need {NCORES} cores, have {len(jax.devices())}")
    mesh = Mesh(np.asarray(devices), ("core",))
    # No donation: the kernel writes every element of `out`, so the seed
    # buffer's contents never matter and a committed zeros array can be
    # reused across calls (XLA copies it device-side for the in-place
    # custom call) — only the image then crosses the axon link per call.
    sharded = jax.jit(
        shard_map(
            _body, mesh=mesh,
            in_specs=(PartitionSpec("core"),) * (n_params + len(out_names)),
            out_specs=(PartitionSpec("core"),) * len(out_names),
            check_rep=False,
        ),
        keep_unused=True,
    )
    sharding = NamedSharding(mesh, PartitionSpec("core"))
    zarrs = [
        jax.device_put(np.zeros((NCORES * s[0], *s[1:]), d), sharding)
        for s, d in zero_shapes
    ]
    return {
        "sharded": sharded, "in_names": in_names, "out_names": out_names,
        "zero_shapes": zero_shapes, "sharding": sharding, "zarrs": zarrs,
    }


def _weights_key(ws):
    h = hashlib.blake2b(digest_size=16)
    for w in ws:
        a = np.ascontiguousarray(np.asarray(w))
        h.update(str(a.shape).encode())
        h.update(a.tobytes())
    return h.hexdigest()


# ---------------------------------------------------------------- entry
def kernel(images, w1, b1, w2, b2, w3, b3, wd, bd, c_x, c_y, comp_w, sigma):
    ws = (w1, b1, w2, b2, w3, b3, wd, bd, c_x, c_y, comp_w, sigma)
    # fast path: same weight objects as last call -> skip hashing
    if "wids" in _CACHE and all(a is b for a, b in zip(_CACHE["wids"], ws)):
        wkey = _CACHE["wkey"]
    else:
        wkey = _weights_key(ws)
    if _CACHE.get("wkey") != wkey:
        consts, _, _ = _build_consts(*ws)
        wb = _weight_blob(consts)                       # [128, W1]
        nc = _build_nc(consts["a2c"].shape[1], consts["a3c"].shape[1], wb)
        _CACHE["exec"] = _make_runner(nc)
        _CACHE["cy"] = np.asarray(c_y, np.float64).copy()
        _CACHE["wkey"] = wkey
    _CACHE["wids"] = ws
    ex = _CACHE["exec"]

    img_glob = _prep_images(images)          # (8, 3, 2, 2, 98, 48, 3)
    args = {"img": img_glob}
    outs = ex["sharded"](*[args[n] for n in ex["in_names"]], *ex["zarrs"])
    oi = ex["out_names"].index("out")
    S8 = np.asarray(outs[oi]).reshape(NCORES, 128, NIMG)
    return _postprocess(S8, _CACHE["cy"])


def _postprocess(S8, cy):
    S = np.zeros((16, 128), np.float64)
    for c in range(NCORES):
        for i in range(NIMG):
            S[NIMG * c + i] = S8[c, :, i]
    S += NPI * EPS
    y_w = S / S.sum(-1, keepdims=True)
    y_v = cy / np.linalg.norm(cy, axis=-1, keepdims=True)
    probs = y_w @ (y_v ** 2)
    return np.ascontiguousarray(probs, F32)
